# revision 1
# baseline (speedup 1.0000x reference)
import sys

import numpy as np

sys.path.insert(0, "/opt/trn_rl_repo")

from concourse import bacc, bass, mybir, tile  # noqa: E402

F16 = mybir.dt.float16
F32 = mybir.dt.float32
TANH = mybir.ActivationFunctionType.Tanh
MULT = mybir.AluOpType.mult
ADD = mybir.AluOpType.add

B, T, C, H = 512, 128, 512, 1024
N_CORES = 8
BC = B // N_CORES  # 64 batch rows per core
CK = C // 128  # 4 feature chunks of y/K
HK = H // 128  # 8 feature chunks of h
YF = CK * BC  # 256 free cols in y-layout tiles
HF = HK * BC  # 512 free cols in h-layout tiles
DT = 1.0 / (T - 1)
UNROLL = 42
N_ITERS = (T - 2) // UNROLL  # steps 2..127 -> 21 iterations of 6


def _mm(nc, out, lhsT, rhs, start, stop):
    nc.tensor.matmul(out, lhsT, rhs, start=start, stop=stop, skip_group_check=True)


def build(n_iters=N_ITERS, unroll=UNROLL, py_loop=False):
    nc = bacc.Bacc("TRN2", target_bir_lowering=False, debug=False,
                   num_devices=N_CORES)

    w1_d = nc.dram_tensor("w1", [128, CK * H], F16, kind="ExternalInput")
    w2_d = nc.dram_tensor("w2", [128, HK * H], F16, kind="ExternalInput")
    w3_d = nc.dram_tensor("w3", [128, HK * C], F16, kind="ExternalInput")
    b1_d = nc.dram_tensor("b1r", [HK, 128], F16, kind="ExternalInput")
    b2_d = nc.dram_tensor("b2r", [HK, 128], F16, kind="ExternalInput")
    b3_d = nc.dram_tensor("b3r", [CK, 128], F16, kind="ExternalInput")
    ind_d = nc.dram_tensor("ind", [CK, YF], F16, kind="ExternalInput")
    y0_d = nc.dram_tensor("y0", [128, YF], F32, kind="ExternalInput")
    y1_d = nc.dram_tensor("y1out", [128, YF], F32, kind="ExternalOutput")
    yo_d = nc.dram_tensor("yout", [n_iters, 128, unroll * YF], F32,
                          kind="ExternalOutput")

    with tile.TileContext(nc) as tc:
        with (
            tc.tile_pool(name="per", bufs=1) as pp,
            tc.tile_pool(name="obuf", bufs=2) as op,
            tc.tile_pool(name="lp", bufs=1, space=bass.MemorySpace.PSUM) as lp,
            tc.tile_pool(name="kp", bufs=1, space=bass.MemorySpace.PSUM) as kp,
        ):
            w1 = pp.tile([128, CK * H], F16)
            w2 = pp.tile([128, HK * H], F16)
            w3 = pp.tile([128, HK * C], F16)
            b1a = pp.tile([CK, 128], F16)
            b1b = pp.tile([CK, 128], F16)
            b2a = pp.tile([CK, 128], F16)
            b2b = pp.tile([CK, 128], F16)
            b3a = pp.tile([CK, 128], F16)
            ind = pp.tile([CK, YF], F16)
            y32 = pp.tile([128, YF], F32)
            y16 = pp.tile([128, YF], F16)
            a2 = pp.tile([128, YF], F16)
            a3 = pp.tile([128, YF], F16)
            a4 = pp.tile([128, YF], F16)
            h1 = pp.tile([128, HF], F16)
            h2 = pp.tile([128, HF], F16)
            p1 = pp.tile([128, YF], F32)
            p2 = pp.tile([128, YF], F32)
            p3 = pp.tile([128, YF], F32)

            nc.sync.dma_start(w1[:], w1_d[:])
            nc.sync.dma_start(w2[:], w2_d[:])
            nc.sync.dma_start(w3[:], w3_d[:])
            nc.sync.dma_start(b1a[:], b1_d[0:CK, :])
            nc.sync.dma_start(b1b[:], b1_d[CK:HK, :])
            nc.sync.dma_start(b2a[:], b2_d[0:CK, :])
            nc.sync.dma_start(b2b[:], b2_d[CK:HK, :])
            nc.sync.dma_start(b3a[:], b3_d[:])
            nc.sync.dma_start(ind[:], ind_d[:])
            nc.sync.dma_start(y32[:], y0_d[:])
            nc.vector.tensor_copy(y16[:], y32[:])

            def feval(arg, kb):
                # layer 1: C=512 in (4 chunks), H=1024 out (8 m) -> banks A,B
                ba = lp.tile([128, 512], F32)
                bb = lp.tile([128, 512], F32)
                _mm(nc, ba[:, 0:YF], b1a[:], ind[:], True, False)
                _mm(nc, bb[:, 0:YF], b1b[:], ind[:], True, False)
                for m in range(4):
                    for k in range(CK):
                        _mm(nc, ba[:, m * BC:(m + 1) * BC],
                            w1[:, k * H + m * 128:k * H + (m + 1) * 128],
                            arg[:, k * BC:(k + 1) * BC], False, k == CK - 1)
                nc.scalar.activation(h1[:, 0:YF], ba[:, 0:YF], TANH)
                for m in range(4):
                    for k in range(CK):
                        _mm(nc, bb[:, m * BC:(m + 1) * BC],
                            w1[:, k * H + (m + 4) * 128:k * H + (m + 5) * 128],
                            arg[:, k * BC:(k + 1) * BC], False, k == CK - 1)
                nc.scalar.activation(h1[:, YF:HF], bb[:, 0:YF], TANH)

                # layer 2: H in (8 chunks, k-outer), H out (8 m) -> banks C,D
                bc_ = lp.tile([128, 512], F32)
                bd = lp.tile([128, 512], F32)
                _mm(nc, bc_[:, 0:YF], b2a[:], ind[:], True, False)
                _mm(nc, bd[:, 0:YF], b2b[:], ind[:], True, False)
                for k in range(HK):
                    for m in range(4):
                        _mm(nc, bc_[:, m * BC:(m + 1) * BC],
                            w2[:, k * H + m * 128:k * H + (m + 1) * 128],
                            h1[:, k * BC:(k + 1) * BC], False, k == HK - 1)
                nc.scalar.activation(h2[:, 0:YF], bc_[:, 0:YF], TANH)
                for k in range(HK):
                    for m in range(4):
                        _mm(nc, bd[:, m * BC:(m + 1) * BC],
                            w2[:, k * H + (m + 4) * 128:k * H + (m + 5) * 128],
                            h1[:, k * BC:(k + 1) * BC], False, k == HK - 1)
                nc.scalar.activation(h2[:, YF:HF], bd[:, 0:YF], TANH)

                # layer 3 (affine, no tanh): H in (8 chunks), C out (4 m) -> kb
                _mm(nc, kb[:, 0:YF], b3a[:], ind[:], True, False)
                for k in range(HK):
                    for m in range(4):
                        _mm(nc, kb[:, m * BC:(m + 1) * BC],
                            w3[:, k * C + m * 128:k * C + (m + 1) * 128],
                            h2[:, k * BC:(k + 1) * BC], False, k == HK - 1)

            def stt(out, in0, s, in1):
                nc.vector.scalar_tensor_tensor(out, in0, float(s), in1, MULT, ADD)

            def step(ybuf_slice):
                k1 = kp.tile([128, 512], F32, name="ka")
                feval(y16[:], k1)
                stt(a2[:], k1[:, 0:YF], 0.5 * DT, y32[:])
                k2 = kp.tile([128, 512], F32, name="kb")
                feval(a2[:], k2)
                stt(p1[:], k1[:, 0:YF], DT / 6, y32[:])
                stt(a3[:], k2[:, 0:YF], 0.5 * DT, y32[:])
                k3 = kp.tile([128, 512], F32, name="ka")
                feval(a3[:], k3)
                stt(p2[:], k2[:, 0:YF], DT / 3, p1[:])
                stt(a4[:], k3[:, 0:YF], DT, y32[:])
                k4 = kp.tile([128, 512], F32, name="kb")
                feval(a4[:], k4)
                stt(p3[:], k3[:, 0:YF], DT / 3, p2[:])
                stt(y16[:], k4[:, 0:YF], DT / 6, p3[:])
                if ybuf_slice is not None:
                    stt(ybuf_slice, k4[:, 0:YF], DT / 6, p3[:])
                stt(y32[:], k4[:, 0:YF], DT / 6, p3[:])

            step(None)
            nc.sync.dma_start(y1_d[:], y32[:])

            def body(it):
                ybuf = op.tile([128, unroll * YF], F32)
                for u in range(unroll):
                    step(ybuf[:, u * YF:(u + 1) * YF])
                nc.sync.dma_start(yo_d[bass.ds(it, 1)], ybuf[:])

            if py_loop:
                for it in range(n_iters):
                    body(it)
            else:
                with tc.For_i(0, n_iters, 1) as it:
                    body(it)

    nc.compile()
    return nc


def _prep_in_maps(x, W1, b1, W2, b2, W3, b3):
    w1 = np.ascontiguousarray(
        W1.reshape(CK, 128, H).transpose(1, 0, 2).reshape(128, CK * H)
    ).astype(np.float16)
    w2 = np.ascontiguousarray(
        W2.reshape(HK, 128, H).transpose(1, 0, 2).reshape(128, HK * H)
    ).astype(np.float16)
    w3 = np.ascontiguousarray(
        W3.reshape(HK, 128, C).transpose(1, 0, 2).reshape(128, HK * C)
    ).astype(np.float16)
    b1r = b1.reshape(HK, 128).astype(np.float16)
    b2r = b2.reshape(HK, 128).astype(np.float16)
    b3r = b3.reshape(CK, 128).astype(np.float16)
    ind = np.zeros((CK, YF), np.float16)
    for k in range(CK):
        ind[k, k * BC:(k + 1) * BC] = 1.0
    shared = dict(w1=w1, w2=w2, w3=w3, b1r=b1r, b2r=b2r, b3r=b3r, ind=ind)
    in_maps = []
    for c in range(N_CORES):
        xs = x[c * BC:(c + 1) * BC, 0, :]  # [BC, C] f32
        y0 = np.ascontiguousarray(
            xs.T.reshape(CK, 128, BC).transpose(1, 0, 2).reshape(128, YF)
        ).astype(np.float32)
        in_maps.append(dict(shared, y0=y0))
    return in_maps


_NC_CACHE = {}


def kernel(**inputs):
    from concourse.bass_utils import run_bass_kernel_spmd

    x = np.asarray(inputs["x"], np.float32)
    in_maps = _prep_in_maps(
        x,
        np.asarray(inputs["W1"], np.float32), np.asarray(inputs["b1"], np.float32),
        np.asarray(inputs["W2"], np.float32), np.asarray(inputs["b2"], np.float32),
        np.asarray(inputs["W3"], np.float32), np.asarray(inputs["b3"], np.float32),
    )
    if "nc" not in _NC_CACHE:
        _NC_CACHE["nc"] = build()
    nc = _NC_CACHE["nc"]

    res = run_bass_kernel_spmd(nc, in_maps, list(range(N_CORES)))
    _NC_CACHE["last_result"] = res

    out = np.empty((B, T, C), np.float32)
    out[:, 0, :] = x[:, 0, :]
    for c in range(N_CORES):
        r = res.results[c]
        rows = slice(c * BC, (c + 1) * BC)
        y1 = np.asarray(r["y1out"], np.float32)
        out[rows, 1, :] = y1.reshape(128, CK, BC).transpose(2, 1, 0).reshape(BC, C)
        yo = np.asarray(r["yout"], np.float32)
        seq = yo.reshape(N_ITERS, 128, UNROLL, CK, BC)
        seq = seq.transpose(0, 2, 4, 3, 1).reshape(T - 2, BC, C)
        out[rows, 2:, :] = seq.transpose(1, 0, 2)
    return out



# revision 2
# speedup vs baseline: 2.5918x; 2.5918x over previous
import sys

import numpy as np

sys.path.insert(0, "/opt/trn_rl_repo")

from concourse import bacc, bass, mybir, tile  # noqa: E402

F16 = mybir.dt.float16
F32 = mybir.dt.float32
TANH = mybir.ActivationFunctionType.Tanh
MULT = mybir.AluOpType.mult
ADD = mybir.AluOpType.add

B, T, C, H = 512, 128, 512, 1024
N_CORES = 8
BC = B // N_CORES  # 64 batch rows per core
CK = C // 128  # 4 feature chunks of y/K
HK = H // 128  # 8 feature chunks of h
YF = CK * BC  # 256 free cols in y-layout tiles
HF = HK * BC  # 512 free cols in h-layout tiles
DT = 1.0 / (T - 1)


def _mm(nc, out, lhsT, rhs, start, stop):
    nc.tensor.matmul(out, lhsT, rhs, start=start, stop=stop, skip_group_check=True)


def build():
    nc = bacc.Bacc("TRN2", target_bir_lowering=False, debug=False,
                   num_devices=N_CORES)

    w1_d = nc.dram_tensor("w1", [128, CK * H], F16, kind="ExternalInput")
    w2_d = nc.dram_tensor("w2", [128, HK * H], F16, kind="ExternalInput")
    w3_d = nc.dram_tensor("w3", [128, HK * C], F16, kind="ExternalInput")
    b1_d = nc.dram_tensor("b1r", [HK, 128], F16, kind="ExternalInput")
    b2_d = nc.dram_tensor("b2r", [HK, 128], F16, kind="ExternalInput")
    b3_d = nc.dram_tensor("b3r", [CK, 128], F16, kind="ExternalInput")
    ind_d = nc.dram_tensor("ind", [CK, YF], F16, kind="ExternalInput")
    y0_d = nc.dram_tensor("y0", [128, YF], F32, kind="ExternalInput")
    yo_d = nc.dram_tensor("yout", [T - 1, 128, YF], F16, kind="ExternalOutput")

    with tile.TileContext(nc) as tc:
        with (
            tc.tile_pool(name="per", bufs=1) as pp,
            tc.tile_pool(name="obuf", bufs=2) as op,
            tc.tile_pool(name="lp", bufs=1, space=bass.MemorySpace.PSUM) as lp,
            tc.tile_pool(name="kp", bufs=1, space=bass.MemorySpace.PSUM) as kp,
        ):
            w1 = pp.tile([128, CK * H], F16)
            w2 = pp.tile([128, HK * H], F16)
            w3 = pp.tile([128, HK * C], F16)
            b1a = pp.tile([CK, 128], F16)
            b1b = pp.tile([CK, 128], F16)
            b2a = pp.tile([CK, 128], F16)
            b2b = pp.tile([CK, 128], F16)
            b3a = pp.tile([CK, 128], F16)
            ind = pp.tile([CK, YF], F16)
            y32 = pp.tile([128, YF], F32)
            y16 = pp.tile([128, YF], F16)
            a2 = pp.tile([128, YF], F16)
            a3 = pp.tile([128, YF], F16)
            a4 = pp.tile([128, YF], F16)
            h1 = pp.tile([128, HF], F16)
            h2 = pp.tile([128, HF], F16)
            p1 = pp.tile([128, YF], F32)
            p2 = pp.tile([128, YF], F32)
            p3 = pp.tile([128, YF], F32)

            nc.sync.dma_start(w1[:], w1_d[:])
            nc.sync.dma_start(w2[:], w2_d[:])
            nc.sync.dma_start(w3[:], w3_d[:])
            nc.sync.dma_start(b1a[:], b1_d[0:CK, :])
            nc.sync.dma_start(b1b[:], b1_d[CK:HK, :])
            nc.sync.dma_start(b2a[:], b2_d[0:CK, :])
            nc.sync.dma_start(b2b[:], b2_d[CK:HK, :])
            nc.sync.dma_start(b3a[:], b3_d[:])
            nc.sync.dma_start(ind[:], ind_d[:])
            nc.sync.dma_start(y32[:], y0_d[:])
            nc.vector.tensor_copy(y16[:], y32[:])

            def feval(arg, kb):
                # layer 1: C=512 in (4 chunks), H=1024 out (8 m) -> banks A,B
                ba = lp.tile([128, 512], F32)
                bb = lp.tile([128, 512], F32)
                _mm(nc, ba[:, 0:YF], b1a[:], ind[:], True, False)
                _mm(nc, bb[:, 0:YF], b1b[:], ind[:], True, False)
                for m in range(4):
                    for k in range(CK):
                        _mm(nc, ba[:, m * BC:(m + 1) * BC],
                            w1[:, k * H + m * 128:k * H + (m + 1) * 128],
                            arg[:, k * BC:(k + 1) * BC], False, k == CK - 1)
                nc.scalar.activation(h1[:, 0:YF], ba[:, 0:YF], TANH)
                for m in range(4):
                    for k in range(CK):
                        _mm(nc, bb[:, m * BC:(m + 1) * BC],
                            w1[:, k * H + (m + 4) * 128:k * H + (m + 5) * 128],
                            arg[:, k * BC:(k + 1) * BC], False, k == CK - 1)
                nc.scalar.activation(h1[:, YF:HF], bb[:, 0:YF], TANH)

                # layer 2: H in (8 chunks, k-outer), H out (8 m) -> banks C,D
                bc_ = lp.tile([128, 512], F32)
                bd = lp.tile([128, 512], F32)
                _mm(nc, bc_[:, 0:YF], b2a[:], ind[:], True, False)
                _mm(nc, bd[:, 0:YF], b2b[:], ind[:], True, False)
                for k in range(HK):
                    for m in range(4):
                        _mm(nc, bc_[:, m * BC:(m + 1) * BC],
                            w2[:, k * H + m * 128:k * H + (m + 1) * 128],
                            h1[:, k * BC:(k + 1) * BC], False, k == HK - 1)
                nc.scalar.activation(h2[:, 0:YF], bc_[:, 0:YF], TANH)
                for k in range(HK):
                    for m in range(4):
                        _mm(nc, bd[:, m * BC:(m + 1) * BC],
                            w2[:, k * H + (m + 4) * 128:k * H + (m + 5) * 128],
                            h1[:, k * BC:(k + 1) * BC], False, k == HK - 1)
                nc.scalar.activation(h2[:, YF:HF], bd[:, 0:YF], TANH)

                # layer 3 (affine, no tanh): H in (8 chunks), C out (4 m) -> kb
                _mm(nc, kb[:, 0:YF], b3a[:], ind[:], True, False)
                for k in range(HK):
                    for m in range(4):
                        _mm(nc, kb[:, m * BC:(m + 1) * BC],
                            w3[:, k * C + m * 128:k * C + (m + 1) * 128],
                            h2[:, k * BC:(k + 1) * BC], False, k == HK - 1)

            def stt(out, in0, s, in1):
                nc.vector.scalar_tensor_tensor(out, in0, float(s), in1, MULT, ADD)

            def step():
                k1 = kp.tile([128, 512], F32, name="ka")
                feval(y16[:], k1)
                stt(a2[:], k1[:, 0:YF], 0.5 * DT, y32[:])
                k2 = kp.tile([128, 512], F32, name="kb")
                feval(a2[:], k2)
                stt(p1[:], k1[:, 0:YF], DT / 6, y32[:])
                stt(a3[:], k2[:, 0:YF], 0.5 * DT, y32[:])
                k3 = kp.tile([128, 512], F32, name="ka")
                feval(a3[:], k3)
                stt(p2[:], k2[:, 0:YF], DT / 3, p1[:])
                stt(a4[:], k3[:, 0:YF], DT, y32[:])
                k4 = kp.tile([128, 512], F32, name="kb")
                feval(a4[:], k4)
                stt(p3[:], k3[:, 0:YF], DT / 3, p2[:])
                stt(y16[:], k4[:, 0:YF], DT / 6, p3[:])
                stt(y32[:], k4[:, 0:YF], DT / 6, p3[:])

            with tc.For_i(0, T - 1, 1) as it:
                step()
                ybuf = op.tile([128, YF], F16)
                nc.vector.tensor_copy(ybuf[:], y16[:])
                nc.sync.dma_start(yo_d[bass.ds(it, 1)], ybuf[:])

    nc.compile()
    return nc


def _prep_in_maps(x, W1, b1, W2, b2, W3, b3):
    w1 = np.ascontiguousarray(
        W1.reshape(CK, 128, H).transpose(1, 0, 2).reshape(128, CK * H)
    ).astype(np.float16)
    w2 = np.ascontiguousarray(
        W2.reshape(HK, 128, H).transpose(1, 0, 2).reshape(128, HK * H)
    ).astype(np.float16)
    w3 = np.ascontiguousarray(
        W3.reshape(HK, 128, C).transpose(1, 0, 2).reshape(128, HK * C)
    ).astype(np.float16)
    b1r = b1.reshape(HK, 128).astype(np.float16)
    b2r = b2.reshape(HK, 128).astype(np.float16)
    b3r = b3.reshape(CK, 128).astype(np.float16)
    ind = np.zeros((CK, YF), np.float16)
    for k in range(CK):
        ind[k, k * BC:(k + 1) * BC] = 1.0
    shared = dict(w1=w1, w2=w2, w3=w3, b1r=b1r, b2r=b2r, b3r=b3r, ind=ind)
    in_maps = []
    for c in range(N_CORES):
        xs = x[c * BC:(c + 1) * BC, 0, :]  # [BC, C] f32
        y0 = np.ascontiguousarray(
            xs.T.reshape(CK, 128, BC).transpose(1, 0, 2).reshape(128, YF)
        ).astype(np.float32)
        in_maps.append(dict(shared, y0=y0))
    return in_maps


_NC_CACHE = {}


def kernel(**inputs):
    from concourse.bass_utils import run_bass_kernel_spmd

    x = np.asarray(inputs["x"], np.float32)
    in_maps = _prep_in_maps(
        x,
        np.asarray(inputs["W1"], np.float32), np.asarray(inputs["b1"], np.float32),
        np.asarray(inputs["W2"], np.float32), np.asarray(inputs["b2"], np.float32),
        np.asarray(inputs["W3"], np.float32), np.asarray(inputs["b3"], np.float32),
    )
    if "nc" not in _NC_CACHE:
        _NC_CACHE["nc"] = build()
    nc = _NC_CACHE["nc"]

    res = run_bass_kernel_spmd(nc, in_maps, list(range(N_CORES)))
    _NC_CACHE["last_result"] = res

    out = np.empty((B, T, C), np.float32)
    out[:, 0, :] = x[:, 0, :]
    for c in range(N_CORES):
        rows = slice(c * BC, (c + 1) * BC)
        yo = np.asarray(res.results[c]["yout"])  # [T-1, 128, YF] f16
        seq = yo.reshape(T - 1, 128, CK, BC)
        # [t, c128, ck, bc] -> [bc, t, ck, c128]; single strided pass + cast
        np.copyto(out[rows, 1:, :].reshape(BC, T - 1, CK, 128),
                  seq.transpose(3, 0, 2, 1))
    return out


# revision 7
# speedup vs baseline: 3.0755x; 1.1866x over previous
import sys

import numpy as np

sys.path.insert(0, "/opt/trn_rl_repo")

from concourse import bacc, bass, mybir, tile  # noqa: E402

F16 = mybir.dt.float16
F32 = mybir.dt.float32
TANH = mybir.ActivationFunctionType.Tanh
MULT = mybir.AluOpType.mult
ADD = mybir.AluOpType.add

B, T, C, H = 512, 128, 512, 1024
N_CORES = 8
BC = B // N_CORES  # 64 batch rows per core
CK = C // 128  # 4 feature chunks of y/K
HK = H // 128  # 8 feature chunks of h
YF = CK * BC  # 256 free cols in y-layout tiles
HF = HK * BC  # 512 free cols in h-layout tiles
DT = 1.0 / (T - 1)
WC = CK * H + HK * H + HK * C  # 16384 combined weight cols
WS = WC // N_CORES  # 2048 cols per core shard


def _mm(nc, out, lhsT, rhs, start, stop):
    nc.tensor.matmul(out, lhsT, rhs, start=start, stop=stop, skip_group_check=True)


def build():
    nc = bacc.Bacc("TRN2", target_bir_lowering=False, debug=False,
                   num_devices=N_CORES)

    # weights are identical on every core: ship 1/8 per core, AllGather on
    # device. Combined [128, 16384] f16 image = w1|w2|w3 cols; core c holds
    # cols [2048c, 2048(c+1)).
    ws_d = nc.dram_tensor("wshard", [128, WS], F16, kind="ExternalInput")
    b1_d = nc.dram_tensor("b1r", [HK, 128], F16, kind="ExternalInput")
    b2_d = nc.dram_tensor("b2r", [HK, 128], F16, kind="ExternalInput")
    b3_d = nc.dram_tensor("b3r", [CK, 128], F16, kind="ExternalInput")
    ind_d = nc.dram_tensor("ind", [CK, YF], F16, kind="ExternalInput")
    y0_d = nc.dram_tensor("y0", [128, YF], F32, kind="ExternalInput")
    yo_d = nc.dram_tensor("yout", [T - 1, 128, YF], F16, kind="ExternalOutput")

    with tile.TileContext(nc) as tc:
        with (
            tc.tile_pool(name="per", bufs=1) as pp,
            tc.tile_pool(name="obuf", bufs=2) as op,
            tc.tile_pool(name="dram", bufs=1, space="DRAM") as dp,
            tc.tile_pool(name="lp", bufs=1, space=bass.MemorySpace.PSUM) as lp,
            tc.tile_pool(name="kp", bufs=1, space=bass.MemorySpace.PSUM) as kp,
        ):
            w1 = pp.tile([128, CK * H], F16)
            w2 = pp.tile([128, HK * H], F16)
            w3 = pp.tile([128, HK * C], F16)
            b1a = pp.tile([CK, 128], F16)
            b1b = pp.tile([CK, 128], F16)
            b2a = pp.tile([CK, 128], F16)
            b2b = pp.tile([CK, 128], F16)
            b3a = pp.tile([CK, 128], F16)
            ind = pp.tile([CK, YF], F16)
            y32 = pp.tile([128, YF], F32)
            y16 = pp.tile([128, YF], F16)
            a2 = pp.tile([128, YF], F16)
            a3 = pp.tile([128, YF], F16)
            a4 = pp.tile([128, YF], F16)
            h1 = pp.tile([128, HF], F16)
            h2 = pp.tile([128, HF], F16)
            p1 = pp.tile([128, YF], F32)
            p2 = pp.tile([128, YF], F32)
            p3 = pp.tile([128, YF], F32)

            wsb = dp.tile([128, WS], F16)
            wg = dp.tile([N_CORES * 128, WS], F16)
            nc.gpsimd.dma_start(wsb[:], ws_d[:])
            nc.gpsimd.collective_compute(
                "AllGather", mybir.AluOpType.bypass,
                replica_groups=[list(range(N_CORES))],
                ins=[wsb.opt()], outs=[wg.opt()])
            # gathered block b = combined cols [WS*b, WS*(b+1)) -> SBUF tiles
            for blk in range(N_CORES):
                col = blk * WS
                if col < CK * H:
                    dst = w1[:, col:col + WS]
                elif col < CK * H + HK * H:
                    dst = w2[:, col - CK * H:col - CK * H + WS]
                else:
                    dst = w3[:, col - CK * H - HK * H:col - CK * H - HK * H + WS]
                nc.sync.dma_start(dst, wg[blk * 128:(blk + 1) * 128, :])
            nc.sync.dma_start(b1a[:], b1_d[0:CK, :])
            nc.sync.dma_start(b1b[:], b1_d[CK:HK, :])
            nc.sync.dma_start(b2a[:], b2_d[0:CK, :])
            nc.sync.dma_start(b2b[:], b2_d[CK:HK, :])
            nc.sync.dma_start(b3a[:], b3_d[:])
            nc.sync.dma_start(ind[:], ind_d[:])
            nc.sync.dma_start(y32[:], y0_d[:])
            nc.vector.tensor_copy(y16[:], y32[:])

            def feval(arg, kb):
                # layer 1: C=512 in (4 chunks), H=1024 out (8 m) -> banks A,B
                ba = lp.tile([128, 512], F32)
                bb = lp.tile([128, 512], F32)
                _mm(nc, ba[:, 0:YF], b1a[:], ind[:], True, False)
                _mm(nc, bb[:, 0:YF], b1b[:], ind[:], True, False)
                for m in range(4):
                    for k in range(CK):
                        _mm(nc, ba[:, m * BC:(m + 1) * BC],
                            w1[:, k * H + m * 128:k * H + (m + 1) * 128],
                            arg[:, k * BC:(k + 1) * BC], False, k == CK - 1)
                nc.scalar.activation(h1[:, 0:YF], ba[:, 0:YF], TANH)
                for m in range(4):
                    for k in range(CK):
                        _mm(nc, bb[:, m * BC:(m + 1) * BC],
                            w1[:, k * H + (m + 4) * 128:k * H + (m + 5) * 128],
                            arg[:, k * BC:(k + 1) * BC], False, k == CK - 1)
                nc.scalar.activation(h1[:, YF:HF], bb[:, 0:YF], TANH)

                # layer 2: H in (8 chunks, k-outer), H out (8 m) -> banks C,D
                bc_ = lp.tile([128, 512], F32)
                bd = lp.tile([128, 512], F32)
                _mm(nc, bc_[:, 0:YF], b2a[:], ind[:], True, False)
                _mm(nc, bd[:, 0:YF], b2b[:], ind[:], True, False)
                for k in range(HK):
                    for m in range(4):
                        _mm(nc, bc_[:, m * BC:(m + 1) * BC],
                            w2[:, k * H + m * 128:k * H + (m + 1) * 128],
                            h1[:, k * BC:(k + 1) * BC], False, k == HK - 1)
                nc.scalar.activation(h2[:, 0:YF], bc_[:, 0:YF], TANH)
                for k in range(HK):
                    for m in range(4):
                        _mm(nc, bd[:, m * BC:(m + 1) * BC],
                            w2[:, k * H + (m + 4) * 128:k * H + (m + 5) * 128],
                            h1[:, k * BC:(k + 1) * BC], False, k == HK - 1)
                nc.scalar.activation(h2[:, YF:HF], bd[:, 0:YF], TANH)

                # layer 3 (affine, no tanh): H in (8 chunks), C out (4 m) -> kb
                _mm(nc, kb[:, 0:YF], b3a[:], ind[:], True, False)
                for k in range(HK):
                    for m in range(4):
                        _mm(nc, kb[:, m * BC:(m + 1) * BC],
                            w3[:, k * C + m * 128:k * C + (m + 1) * 128],
                            h2[:, k * BC:(k + 1) * BC], False, k == HK - 1)

            def stt(out, in0, s, in1):
                nc.vector.scalar_tensor_tensor(out, in0, float(s), in1, MULT, ADD)

            def step():
                k1 = kp.tile([128, 512], F32, name="ka")
                feval(y16[:], k1)
                stt(a2[:], k1[:, 0:YF], 0.5 * DT, y32[:])
                k2 = kp.tile([128, 512], F32, name="kb")
                feval(a2[:], k2)
                stt(p1[:], k1[:, 0:YF], DT / 6, y32[:])
                stt(a3[:], k2[:, 0:YF], 0.5 * DT, y32[:])
                k3 = kp.tile([128, 512], F32, name="ka")
                feval(a3[:], k3)
                stt(p2[:], k2[:, 0:YF], DT / 3, p1[:])
                stt(a4[:], k3[:, 0:YF], DT, y32[:])
                k4 = kp.tile([128, 512], F32, name="kb")
                feval(a4[:], k4)
                stt(p3[:], k3[:, 0:YF], DT / 3, p2[:])
                stt(y16[:], k4[:, 0:YF], DT / 6, p3[:])
                stt(y32[:], k4[:, 0:YF], DT / 6, p3[:])

            with tc.For_i(0, T - 1, 1) as it:
                step()
                ybuf = op.tile([128, YF], F16)
                nc.vector.tensor_copy(ybuf[:], y16[:])
                nc.sync.dma_start(yo_d[bass.ds(it, 1)], ybuf[:])

    nc.compile()
    return nc


def _prep_in_maps(x, W1, b1, W2, b2, W3, b3):
    w1 = np.ascontiguousarray(
        W1.reshape(CK, 128, H).transpose(1, 0, 2).reshape(128, CK * H)
    ).astype(np.float16)
    w2 = np.ascontiguousarray(
        W2.reshape(HK, 128, H).transpose(1, 0, 2).reshape(128, HK * H)
    ).astype(np.float16)
    w3 = np.ascontiguousarray(
        W3.reshape(HK, 128, C).transpose(1, 0, 2).reshape(128, HK * C)
    ).astype(np.float16)
    b1r = b1.reshape(HK, 128).astype(np.float16)
    b2r = b2.reshape(HK, 128).astype(np.float16)
    b3r = b3.reshape(CK, 128).astype(np.float16)
    ind = np.zeros((CK, YF), np.float16)
    for k in range(CK):
        ind[k, k * BC:(k + 1) * BC] = 1.0
    wcat = np.concatenate([w1, w2, w3], axis=1)  # [128, WC]
    shared = dict(b1r=b1r, b2r=b2r, b3r=b3r, ind=ind)
    in_maps = []
    for c in range(N_CORES):
        xs = x[c * BC:(c + 1) * BC, 0, :]  # [BC, C] f32
        y0 = np.ascontiguousarray(
            xs.T.reshape(CK, 128, BC).transpose(1, 0, 2).reshape(128, YF)
        ).astype(np.float32)
        in_maps.append(dict(shared, y0=y0,
                            wshard=wcat[:, c * WS:(c + 1) * WS]))
    return in_maps


_NC_CACHE = {}


def kernel(**inputs):
    from concourse.bass_utils import run_bass_kernel_spmd

    x = np.asarray(inputs["x"], np.float32)
    in_maps = _prep_in_maps(
        x,
        np.asarray(inputs["W1"], np.float32), np.asarray(inputs["b1"], np.float32),
        np.asarray(inputs["W2"], np.float32), np.asarray(inputs["b2"], np.float32),
        np.asarray(inputs["W3"], np.float32), np.asarray(inputs["b3"], np.float32),
    )
    if "nc" not in _NC_CACHE:
        _NC_CACHE["nc"] = build()
    nc = _NC_CACHE["nc"]

    res = run_bass_kernel_spmd(nc, in_maps, list(range(N_CORES)))
    _NC_CACHE["last_result"] = res

    out = np.empty((B, T, C), np.float32)
    out[:, 0, :] = x[:, 0, :]
    for c in range(N_CORES):
        rows = slice(c * BC, (c + 1) * BC)
        yo = np.asarray(res.results[c]["yout"])  # [T-1, 128, YF] f16
        seq = yo.reshape(T - 1, 128, CK, BC)
        # [t, c128, ck, bc] -> [bc, t, ck, c128]; single strided pass + cast
        np.copyto(out[rows, 1:, :].reshape(BC, T - 1, CK, 128),
                  seq.transpose(3, 0, 2, 1))
    return out


# revision 12
# speedup vs baseline: 3.3779x; 1.0983x over previous
import sys

import numpy as np

sys.path.insert(0, "/opt/trn_rl_repo")

from concourse import bacc, bass, mybir, tile  # noqa: E402

F16 = mybir.dt.float16
F32 = mybir.dt.float32
F8 = mybir.dt.float8e4
TANH = mybir.ActivationFunctionType.Tanh
COPY = mybir.ActivationFunctionType.Copy
MULT = mybir.AluOpType.mult
ADD = mybir.AluOpType.add
SC = 256.0  # fp8 delta scale: |SC*dy| stays well inside e4m3 range

B, T, C, H = 512, 128, 512, 1024
N_CORES = 8
BC = B // N_CORES  # 64 batch rows per core
CK = C // 128  # 4 feature chunks of y/K
HK = H // 128  # 8 feature chunks of h
YF = CK * BC  # 256 free cols in y-layout tiles
HF = HK * BC  # 512 free cols in h-layout tiles
DT = 1.0 / (T - 1)
WC = CK * H + HK * H + HK * C  # 16384 combined weight cols
WS = WC // N_CORES  # 2048 cols per core shard


def _mm(nc, out, lhsT, rhs, start, stop):
    nc.tensor.matmul(out, lhsT, rhs, start=start, stop=stop, skip_group_check=True)


def build():
    nc = bacc.Bacc("TRN2", target_bir_lowering=False, debug=False,
                   num_devices=N_CORES)

    # weights are identical on every core: ship 1/8 per core, AllGather on
    # device. Combined [128, 16384] f16 image = w1|w2|w3 cols; core c holds
    # cols [2048c, 2048(c+1)).
    ws_d = nc.dram_tensor("wshard", [128, WS], F16, kind="ExternalInput")
    b1_d = nc.dram_tensor("b1r", [HK, 128], F16, kind="ExternalInput")
    b2_d = nc.dram_tensor("b2r", [HK, 128], F16, kind="ExternalInput")
    b3_d = nc.dram_tensor("b3r", [CK, 128], F16, kind="ExternalInput")
    ind_d = nc.dram_tensor("ind", [CK, YF], F16, kind="ExternalInput")
    y0_d = nc.dram_tensor("y0", [128, YF], F32, kind="ExternalInput")
    yo_d = nc.dram_tensor("yout", [T - 1, 128, YF], F8, kind="ExternalOutput")

    with tile.TileContext(nc) as tc:
        with (
            tc.tile_pool(name="per", bufs=1) as pp,
            tc.tile_pool(name="obuf", bufs=2) as op,
            tc.tile_pool(name="dram", bufs=1, space="DRAM") as dp,
            tc.tile_pool(name="lp", bufs=1, space=bass.MemorySpace.PSUM) as lp,
            tc.tile_pool(name="kp", bufs=1, space=bass.MemorySpace.PSUM) as kp,
        ):
            w1 = pp.tile([128, CK * H], F16)
            w2 = pp.tile([128, HK * H], F16)
            w3 = pp.tile([128, HK * C], F16)
            b1a = pp.tile([CK, 128], F16)
            b1b = pp.tile([CK, 128], F16)
            b2a = pp.tile([CK, 128], F16)
            b2b = pp.tile([CK, 128], F16)
            b3a = pp.tile([CK, 128], F16)
            ind = pp.tile([CK, YF], F16)
            y32 = pp.tile([128, YF], F32)
            y16 = pp.tile([128, YF], F16)
            a2 = pp.tile([128, YF], F16)
            a3 = pp.tile([128, YF], F16)
            a4 = pp.tile([128, YF], F16)
            h1 = pp.tile([128, HF], F16)
            h2 = pp.tile([128, HF], F16)
            q1 = pp.tile([128, YF], F32)
            q2 = pp.tile([128, YF], F32)
            q3 = pp.tile([128, YF], F32)
            dsc = pp.tile([128, YF], F32)

            wsb = dp.tile([128, WS], F16)
            wg = dp.tile([N_CORES * 128, WS], F16)
            nc.gpsimd.dma_start(wsb[:], ws_d[:])
            nc.gpsimd.collective_compute(
                "AllGather", mybir.AluOpType.bypass,
                replica_groups=[list(range(N_CORES))],
                ins=[wsb.opt()], outs=[wg.opt()])
            # gathered block b = combined cols [WS*b, WS*(b+1)) -> SBUF tiles
            for blk in range(N_CORES):
                col = blk * WS
                if col < CK * H:
                    dst = w1[:, col:col + WS]
                elif col < CK * H + HK * H:
                    dst = w2[:, col - CK * H:col - CK * H + WS]
                else:
                    dst = w3[:, col - CK * H - HK * H:col - CK * H - HK * H + WS]
                nc.sync.dma_start(dst, wg[blk * 128:(blk + 1) * 128, :])
            nc.sync.dma_start(b1a[:], b1_d[0:CK, :])
            nc.sync.dma_start(b1b[:], b1_d[CK:HK, :])
            nc.sync.dma_start(b2a[:], b2_d[0:CK, :])
            nc.sync.dma_start(b2b[:], b2_d[CK:HK, :])
            nc.sync.dma_start(b3a[:], b3_d[:])
            nc.sync.dma_start(ind[:], ind_d[:])
            nc.sync.dma_start(y32[:], y0_d[:])
            nc.vector.tensor_copy(y16[:], y32[:])

            def feval(arg, kb):
                # layer 1: C=512 in (4 chunks), H=1024 out (8 m) -> banks A,B
                ba = lp.tile([128, 512], F32)
                bb = lp.tile([128, 512], F32)
                _mm(nc, ba[:, 0:YF], b1a[:], ind[:], True, False)
                _mm(nc, bb[:, 0:YF], b1b[:], ind[:], True, False)
                for m in range(4):
                    for k in range(CK):
                        _mm(nc, ba[:, m * BC:(m + 1) * BC],
                            w1[:, k * H + m * 128:k * H + (m + 1) * 128],
                            arg[:, k * BC:(k + 1) * BC], False, k == CK - 1)
                nc.scalar.activation(h1[:, 0:YF], ba[:, 0:YF], TANH)
                for m in range(4):
                    for k in range(CK):
                        _mm(nc, bb[:, m * BC:(m + 1) * BC],
                            w1[:, k * H + (m + 4) * 128:k * H + (m + 5) * 128],
                            arg[:, k * BC:(k + 1) * BC], False, k == CK - 1)
                nc.scalar.activation(h1[:, YF:HF], bb[:, 0:YF], TANH)

                # layer 2: H in (8 chunks, k-outer), H out (8 m) -> banks C,D
                bc_ = lp.tile([128, 512], F32)
                bd = lp.tile([128, 512], F32)
                _mm(nc, bc_[:, 0:YF], b2a[:], ind[:], True, False)
                _mm(nc, bd[:, 0:YF], b2b[:], ind[:], True, False)
                for k in range(HK):
                    for m in range(4):
                        _mm(nc, bc_[:, m * BC:(m + 1) * BC],
                            w2[:, k * H + m * 128:k * H + (m + 1) * 128],
                            h1[:, k * BC:(k + 1) * BC], False, k == HK - 1)
                nc.scalar.activation(h2[:, 0:YF], bc_[:, 0:YF], TANH)
                for k in range(HK):
                    for m in range(4):
                        _mm(nc, bd[:, m * BC:(m + 1) * BC],
                            w2[:, k * H + (m + 4) * 128:k * H + (m + 5) * 128],
                            h1[:, k * BC:(k + 1) * BC], False, k == HK - 1)
                nc.scalar.activation(h2[:, YF:HF], bd[:, 0:YF], TANH)

                # layer 3 (affine, no tanh): H in (8 chunks), C out (4 m) -> kb
                _mm(nc, kb[:, 0:YF], b3a[:], ind[:], True, False)
                for k in range(HK):
                    for m in range(4):
                        _mm(nc, kb[:, m * BC:(m + 1) * BC],
                            w3[:, k * C + m * 128:k * C + (m + 1) * 128],
                            h2[:, k * BC:(k + 1) * BC], False, k == HK - 1)

            def stt(out, in0, s, in1):
                nc.vector.scalar_tensor_tensor(out, in0, float(s), in1, MULT, ADD)

            def step():
                # dsc accumulates SC * (RK4 increment); y += dsc/SC
                k1 = kp.tile([128, 512], F32, name="ka")
                feval(y16[:], k1)
                stt(a2[:], k1[:, 0:YF], 0.5 * DT, y32[:])
                k2 = kp.tile([128, 512], F32, name="kb")
                feval(a2[:], k2)
                nc.scalar.activation(q1[:], k1[:, 0:YF], COPY, scale=SC * DT / 6)
                stt(a3[:], k2[:, 0:YF], 0.5 * DT, y32[:])
                k3 = kp.tile([128, 512], F32, name="ka")
                feval(a3[:], k3)
                stt(q2[:], k2[:, 0:YF], SC * DT / 3, q1[:])
                stt(a4[:], k3[:, 0:YF], DT, y32[:])
                k4 = kp.tile([128, 512], F32, name="kb")
                feval(a4[:], k4)
                stt(q3[:], k3[:, 0:YF], SC * DT / 3, q2[:])
                stt(dsc[:], k4[:, 0:YF], SC * DT / 6, q3[:])
                stt(y16[:], dsc[:], 1.0 / SC, y32[:])
                stt(y32[:], dsc[:], 1.0 / SC, y32[:])

            with tc.For_i(0, T - 1, 1) as it:
                step()
                ybuf = op.tile([128, YF], F8)
                nc.scalar.activation(ybuf[:], dsc[:], COPY)
                nc.sync.dma_start(yo_d[bass.ds(it, 1)], ybuf[:])

    nc.compile()
    return nc


def _prep_in_maps(x, W1, b1, W2, b2, W3, b3):
    w1 = np.ascontiguousarray(
        W1.reshape(CK, 128, H).transpose(1, 0, 2).reshape(128, CK * H)
    ).astype(np.float16)
    w2 = np.ascontiguousarray(
        W2.reshape(HK, 128, H).transpose(1, 0, 2).reshape(128, HK * H)
    ).astype(np.float16)
    w3 = np.ascontiguousarray(
        W3.reshape(HK, 128, C).transpose(1, 0, 2).reshape(128, HK * C)
    ).astype(np.float16)
    b1r = b1.reshape(HK, 128).astype(np.float16)
    b2r = b2.reshape(HK, 128).astype(np.float16)
    b3r = b3.reshape(CK, 128).astype(np.float16)
    ind = np.zeros((CK, YF), np.float16)
    for k in range(CK):
        ind[k, k * BC:(k + 1) * BC] = 1.0
    wcat = np.concatenate([w1, w2, w3], axis=1)  # [128, WC]
    shared = dict(b1r=b1r, b2r=b2r, b3r=b3r, ind=ind)
    in_maps = []
    for c in range(N_CORES):
        xs = x[c * BC:(c + 1) * BC, 0, :]  # [BC, C] f32
        y0 = np.ascontiguousarray(
            xs.T.reshape(CK, 128, BC).transpose(1, 0, 2).reshape(128, YF)
        ).astype(np.float32)
        in_maps.append(dict(shared, y0=y0,
                            wshard=wcat[:, c * WS:(c + 1) * WS]))
    return in_maps


_NC_CACHE = {}


def kernel(**inputs):
    from concourse.bass_utils import run_bass_kernel_spmd

    x = np.asarray(inputs["x"], np.float32)
    in_maps = _prep_in_maps(
        x,
        np.asarray(inputs["W1"], np.float32), np.asarray(inputs["b1"], np.float32),
        np.asarray(inputs["W2"], np.float32), np.asarray(inputs["b2"], np.float32),
        np.asarray(inputs["W3"], np.float32), np.asarray(inputs["b3"], np.float32),
    )
    if "nc" not in _NC_CACHE:
        _NC_CACHE["nc"] = build()
    nc = _NC_CACHE["nc"]

    res = run_bass_kernel_spmd(nc, in_maps, list(range(N_CORES)))
    _NC_CACHE["last_result"] = res

    out = np.empty((B, T, C), np.float32)
    out[:, 0, :] = x[:, 0, :]
    for c in range(N_CORES):
        rows = slice(c * BC, (c + 1) * BC)
        # fp8 scaled per-step deltas -> y_t = y0 + cumsum(delta)/SC
        d = np.asarray(res.results[c]["yout"]).astype(np.float32)  # [T-1,128,YF]
        np.cumsum(d, axis=0, out=d)
        d *= 1.0 / SC
        d += in_maps[c]["y0"][None]
        seq = d.reshape(T - 1, 128, CK, BC)
        # [t, c128, ck, bc] -> [bc, t, ck, c128]; single strided pass
        np.copyto(out[rows, 1:, :].reshape(BC, T - 1, CK, 128),
                  seq.transpose(3, 0, 2, 1))
    return out


# revision 18
# speedup vs baseline: 5.2689x; 1.5598x over previous
import sys

import numpy as np

sys.path.insert(0, "/opt/trn_rl_repo")

from concourse import bacc, bass, mybir, tile  # noqa: E402

F16 = mybir.dt.float16
F32 = mybir.dt.float32
F8 = mybir.dt.float8e4
TANH = mybir.ActivationFunctionType.Tanh
COPY = mybir.ActivationFunctionType.Copy
MULT = mybir.AluOpType.mult
ADD = mybir.AluOpType.add
SC = 256.0  # fp8 delta scale: |SC*dy| stays well inside e4m3 range

B, T, C, H = 512, 128, 512, 1024
N_CORES = 8
BC = B // N_CORES  # 64 batch rows per core
CK = C // 128  # 4 feature chunks of y/K
HK = H // 128  # 8 feature chunks of h
YF = CK * BC  # 256 free cols in y-layout tiles
HF = HK * BC  # 512 free cols in h-layout tiles
DT = 1.0 / (T - 1)
WC = CK * H + HK * H + HK * C  # 16384 combined weight cols
WS = WC // N_CORES  # 2048 cols per core shard


def _mm(nc, out, lhsT, rhs, start, stop):
    nc.tensor.matmul(out, lhsT, rhs, start=start, stop=stop, skip_group_check=True)


def build():
    nc = bacc.Bacc("TRN2", target_bir_lowering=False, debug=False,
                   num_devices=N_CORES)

    # weights are identical on every core: ship 1/8 per core, AllGather on
    # device. Combined [128, 16384] f16 image = w1|w2|w3 cols; core c holds
    # cols [2048c, 2048(c+1)).
    ws_d = nc.dram_tensor("wshard", [128, WS], F16, kind="ExternalInput")
    b1_d = nc.dram_tensor("b1r", [HK, 128], F16, kind="ExternalInput")
    b2_d = nc.dram_tensor("b2r", [HK, 128], F16, kind="ExternalInput")
    b3_d = nc.dram_tensor("b3r", [CK, 128], F16, kind="ExternalInput")
    ind_d = nc.dram_tensor("ind", [CK, YF], F16, kind="ExternalInput")
    idm_d = nc.dram_tensor("idm", [128, 128], F32, kind="ExternalInput")
    y0_d = nc.dram_tensor("y0", [128, YF], F32, kind="ExternalInput")
    # per-core output, host-contiguous [bc, t, c] fp8 scaled deltas
    yo_d = nc.dram_tensor("yout", [BC, (T - 1) * C], F8, kind="ExternalOutput")

    with tile.TileContext(nc) as tc:
        with (
            tc.tile_pool(name="per", bufs=1) as pp,
            tc.tile_pool(name="obuf", bufs=2) as op,
            tc.tile_pool(name="dram", bufs=1, space="DRAM") as dp,
            tc.tile_pool(name="lp", bufs=1, space=bass.MemorySpace.PSUM) as lp,
            tc.tile_pool(name="kp", bufs=1, space=bass.MemorySpace.PSUM) as kp,
        ):
            w1 = pp.tile([128, CK * H], F16)
            w2 = pp.tile([128, HK * H], F16)
            w3 = pp.tile([128, HK * C], F16)
            b1a = pp.tile([CK, 128], F16)
            b1b = pp.tile([CK, 128], F16)
            b2a = pp.tile([CK, 128], F16)
            b2b = pp.tile([CK, 128], F16)
            b3a = pp.tile([CK, 128], F16)
            ind = pp.tile([CK, YF], F16)
            idm = pp.tile([128, 128], F32)
            y32 = pp.tile([128, YF], F32)
            y16 = pp.tile([128, YF], F16)
            a2 = pp.tile([128, YF], F16)
            a3 = pp.tile([128, YF], F16)
            a4 = pp.tile([128, YF], F16)
            h1 = pp.tile([128, HF], F16)
            h2 = pp.tile([128, HF], F16)
            q1 = pp.tile([128, YF], F32)
            q2 = pp.tile([128, YF], F32)
            q3 = pp.tile([128, YF], F32)
            dsc = pp.tile([128, YF], F32)

            wsb = dp.tile([128, WS], F16)
            wg = dp.tile([N_CORES * 128, WS], F16)
            nc.gpsimd.dma_start(wsb[:], ws_d[:])
            nc.gpsimd.collective_compute(
                "AllGather", mybir.AluOpType.bypass,
                replica_groups=[list(range(N_CORES))],
                ins=[wsb.opt()], outs=[wg.opt()])
            # gathered block b = combined cols [WS*b, WS*(b+1)) -> SBUF tiles
            for blk in range(N_CORES):
                col = blk * WS
                if col < CK * H:
                    dst = w1[:, col:col + WS]
                elif col < CK * H + HK * H:
                    dst = w2[:, col - CK * H:col - CK * H + WS]
                else:
                    dst = w3[:, col - CK * H - HK * H:col - CK * H - HK * H + WS]
                nc.sync.dma_start(dst, wg[blk * 128:(blk + 1) * 128, :])
            nc.sync.dma_start(b1a[:], b1_d[0:CK, :])
            nc.sync.dma_start(b1b[:], b1_d[CK:HK, :])
            nc.sync.dma_start(b2a[:], b2_d[0:CK, :])
            nc.sync.dma_start(b2b[:], b2_d[CK:HK, :])
            nc.sync.dma_start(b3a[:], b3_d[:])
            nc.sync.dma_start(ind[:], ind_d[:])
            nc.sync.dma_start(idm[:], idm_d[:])
            nc.sync.dma_start(y32[:], y0_d[:])
            nc.vector.tensor_copy(y16[:], y32[:])

            def feval(arg, kb):
                # layer 1: C=512 in (4 chunks), H=1024 out (8 m) -> banks A,B
                ba = lp.tile([128, 512], F32)
                bb = lp.tile([128, 512], F32)
                _mm(nc, ba[:, 0:YF], b1a[:], ind[:], True, False)
                _mm(nc, bb[:, 0:YF], b1b[:], ind[:], True, False)
                for m in range(4):
                    for k in range(CK):
                        _mm(nc, ba[:, m * BC:(m + 1) * BC],
                            w1[:, k * H + m * 128:k * H + (m + 1) * 128],
                            arg[:, k * BC:(k + 1) * BC], False, k == CK - 1)
                nc.scalar.activation(h1[:, 0:YF], ba[:, 0:YF], TANH)
                for m in range(4):
                    for k in range(CK):
                        _mm(nc, bb[:, m * BC:(m + 1) * BC],
                            w1[:, k * H + (m + 4) * 128:k * H + (m + 5) * 128],
                            arg[:, k * BC:(k + 1) * BC], False, k == CK - 1)
                nc.scalar.activation(h1[:, YF:HF], bb[:, 0:YF], TANH)

                # layer 2: H in (8 chunks, k-outer), H out (8 m) -> banks C,D
                bc_ = lp.tile([128, 512], F32)
                bd = lp.tile([128, 512], F32)
                _mm(nc, bc_[:, 0:YF], b2a[:], ind[:], True, False)
                _mm(nc, bd[:, 0:YF], b2b[:], ind[:], True, False)
                for k in range(HK):
                    for m in range(4):
                        _mm(nc, bc_[:, m * BC:(m + 1) * BC],
                            w2[:, k * H + m * 128:k * H + (m + 1) * 128],
                            h1[:, k * BC:(k + 1) * BC], False, k == HK - 1)
                nc.scalar.activation(h2[:, 0:YF], bc_[:, 0:YF], TANH)
                for k in range(HK):
                    for m in range(4):
                        _mm(nc, bd[:, m * BC:(m + 1) * BC],
                            w2[:, k * H + (m + 4) * 128:k * H + (m + 5) * 128],
                            h1[:, k * BC:(k + 1) * BC], False, k == HK - 1)
                nc.scalar.activation(h2[:, YF:HF], bd[:, 0:YF], TANH)

                # layer 3 (affine, no tanh): H in (8 chunks), C out (4 m) -> kb
                _mm(nc, kb[:, 0:YF], b3a[:], ind[:], True, False)
                for k in range(HK):
                    for m in range(4):
                        _mm(nc, kb[:, m * BC:(m + 1) * BC],
                            w3[:, k * C + m * 128:k * C + (m + 1) * 128],
                            h2[:, k * BC:(k + 1) * BC], False, k == HK - 1)

            def stt(out, in0, s, in1):
                nc.vector.scalar_tensor_tensor(out, in0, float(s), in1, MULT, ADD)

            def step():
                # dsc accumulates SC * (RK4 increment); y += dsc/SC
                k1 = kp.tile([128, 512], F32, name="ka")
                feval(y16[:], k1)
                stt(a2[:], k1[:, 0:YF], 0.5 * DT, y32[:])
                k2 = kp.tile([128, 512], F32, name="kb")
                feval(a2[:], k2)
                nc.scalar.activation(q1[:], k1[:, 0:YF], COPY, scale=SC * DT / 6)
                stt(a3[:], k2[:, 0:YF], 0.5 * DT, y32[:])
                k3 = kp.tile([128, 512], F32, name="ka")
                feval(a3[:], k3)
                stt(q2[:], k2[:, 0:YF], SC * DT / 3, q1[:])
                stt(a4[:], k3[:, 0:YF], DT, y32[:])
                k4 = kp.tile([128, 512], F32, name="kb")
                feval(a4[:], k4)
                stt(q3[:], k3[:, 0:YF], SC * DT / 3, q2[:])
                stt(dsc[:], k4[:, 0:YF], SC * DT / 6, q3[:])
                stt(y16[:], dsc[:], 1.0 / SC, y32[:])
                stt(y32[:], dsc[:], 1.0 / SC, y32[:])

            with tc.For_i(0, (T - 1) * C, C) as it:
                step()
                # transpose dsc [c128,(ck,bc)] -> [bc, c] so dram rows are
                # host-contiguous 512B runs
                tp = kp.tile([BC, C], F32, name="tp")
                for ck in range(CK):
                    nc.tensor.transpose(
                        tp[:, ck * 128:(ck + 1) * 128],
                        dsc[:, ck * BC:(ck + 1) * BC], idm[:])
                ybuf = op.tile([BC, C], F8)
                nc.scalar.activation(ybuf[:], tp[:], COPY)
                nc.sync.dma_start(yo_d[:, bass.ds(it, C)], ybuf[:])

    nc.compile()
    return nc


def _prep_in_maps(x, W1, b1, W2, b2, W3, b3):
    w1 = np.ascontiguousarray(
        W1.reshape(CK, 128, H).transpose(1, 0, 2).reshape(128, CK * H)
    ).astype(np.float16)
    w2 = np.ascontiguousarray(
        W2.reshape(HK, 128, H).transpose(1, 0, 2).reshape(128, HK * H)
    ).astype(np.float16)
    w3 = np.ascontiguousarray(
        W3.reshape(HK, 128, C).transpose(1, 0, 2).reshape(128, HK * C)
    ).astype(np.float16)
    b1r = b1.reshape(HK, 128).astype(np.float16)
    b2r = b2.reshape(HK, 128).astype(np.float16)
    b3r = b3.reshape(CK, 128).astype(np.float16)
    ind = np.zeros((CK, YF), np.float16)
    for k in range(CK):
        ind[k, k * BC:(k + 1) * BC] = 1.0
    wcat = np.concatenate([w1, w2, w3], axis=1)  # [128, WC]
    idm = np.eye(128, dtype=np.float32)
    shared = dict(b1r=b1r, b2r=b2r, b3r=b3r, ind=ind, idm=idm)
    in_maps = []
    for c in range(N_CORES):
        xs = x[c * BC:(c + 1) * BC, 0, :]  # [BC, C] f32
        y0 = np.ascontiguousarray(
            xs.T.reshape(CK, 128, BC).transpose(1, 0, 2).reshape(128, YF)
        ).astype(np.float32)
        in_maps.append(dict(shared, y0=y0,
                            wshard=wcat[:, c * WS:(c + 1) * WS]))
    return in_maps


_NC_CACHE = {}


def kernel(**inputs):
    from concourse.bass_utils import run_bass_kernel_spmd

    x = np.asarray(inputs["x"], np.float32)
    in_maps = _prep_in_maps(
        x,
        np.asarray(inputs["W1"], np.float32), np.asarray(inputs["b1"], np.float32),
        np.asarray(inputs["W2"], np.float32), np.asarray(inputs["b2"], np.float32),
        np.asarray(inputs["W3"], np.float32), np.asarray(inputs["b3"], np.float32),
    )
    if "nc" not in _NC_CACHE:
        _NC_CACHE["nc"] = build()
    nc = _NC_CACHE["nc"]

    res = run_bass_kernel_spmd(nc, in_maps, list(range(N_CORES)))
    _NC_CACHE["last_result"] = res

    out = np.empty((B, T, C), np.float32)
    out[:, 0, :] = x[:, 0, :]
    for c in range(N_CORES):
        rows = slice(c * BC, (c + 1) * BC)
        # fp8 scaled per-step deltas, already [bc, t, c] on device:
        # y_t = y0 + cumsum(delta)/SC, accumulated straight into out
        d8 = np.asarray(res.results[c]["yout"]).reshape(BC, T - 1, C)
        view = out[rows, 1:, :]
        np.cumsum(d8, axis=1, dtype=np.float32, out=view)
        view *= 1.0 / SC
        view += x[rows, 0:1, :]
    return out
